# revision 7
# baseline (speedup 1.0000x reference)
"""DGCNN-alt Trainium2 kernel: 8-core data-parallel (4 graphs/core).

Self-contained: builds a Bass/Tile kernel (weights inlined into the NEFF as
constants), shards `pos` across 8 NeuronCores, runs via PJRT (axon), gathers
the full [32, 40] output.

kNN top-k is exact: f32 -d^2/2 scores + DVE max/match_replace/max_index
(top-24 descending, first-occurrence tie-break = reference tie-break).
"""
import sys
sys.path.insert(0, '/opt/trn_rl_repo')
import hashlib
import numpy as np

import concourse.bass as bass
from concourse import bacc
import concourse.mybir as mybir
from concourse.tile import TileContext
from concourse.bass import IndirectOffsetOnAxis

f32 = mybir.dt.float32
f32r = mybir.dt.float32r
u32 = mybir.dt.uint32
AF = mybir.ActivationFunctionType
ALU = mybir.AluOpType

# ---- problem constants ----
B, N, D, K = 32, 1024, 3, 20
GPC = 4                 # graphs per core
NCORES = 8
EPS = 1e-5
NEDGE = N * K           # 20480 edges/graph
M_EDGES = float(B * NEDGE)   # BN denominator over the full batch
NEG = -3.0e38


def _build(consts):
    nc = bacc.Bacc()

    # ---------------- I/O ----------------
    pos4 = nc.dram_tensor("pos4", [GPC * N, D], f32, kind="ExternalInput")
    out_t = nc.dram_tensor("out", [40, GPC], f32, kind="ExternalOutput")

    # weights/constants baked into the executable
    CD = {name: nc.inline_tensor(np.ascontiguousarray(arr), name=name)
          for name, arr in consts.items()}

    # internal DRAM
    v2d = [nc.dram_tensor(f"v2d_{g}", [N, 128], f32) for g in range(GPC)]
    cc1_in = nc.dram_tensor("cc1_in", [1, 128], f32)
    cc1_out = nc.dram_tensor("cc1_out", [1, 128], f32, addr_space="Shared")
    cc2_in = nc.dram_tensor("cc2_in", [1, 128], f32)
    cc2_out = nc.dram_tensor("cc2_out", [1, 128], f32, addr_space="Shared")
    rg = [list(range(NCORES))]

    with TileContext(nc) as tc:
        with tc.tile_pool(name="cst", bufs=1) as cst, \
             tc.tile_pool(name="big", bufs=1) as big, \
             tc.tile_pool(name="wrk", bufs=2) as wrk, \
             tc.tile_pool(name="sm", bufs=1) as sm, \
             tc.tile_pool(name="psA", bufs=1, space="PSUM") as psA, \
             tc.tile_pool(name="psB", bufs=2, space="PSUM") as psB, \
             tc.tile_pool(name="psC", bufs=2, space="PSUM") as psC:

            # ---------- load constants ----------
            def ld(name, dtype):
                arr = consts[name]
                t = cst.tile(list(arr.shape), dtype, name=name + "T")
                src = CD[name][:, :]
                if dtype != f32:
                    src = src.bitcast(dtype)
                nc.sync.dma_start(out=t, in_=src)
                return t

            RT = ld("Rsel", f32r)
            idT = ld("ident", f32)
            W1bT = ld("W1b_i", f32r)
            W1dT = ld("W1d_i", f32)
            W2T = ld("W2_i", f32r)
            W3T = ld("W3_i", f32r)
            Wc2dT = ld("Wc2d_i", f32r)
            Wc2bT = ld("Wc2b_i", f32r)
            WlX1T = ld("WlX1_i", f32r)
            WlX2T = ld("WlX2_i", f32r)
            b3cT = ld("b3c_i", f32)
            bc2cT = ld("bc2c_i", f32)
            blcT = ld("blc_i", f32)
            g1rT = ld("g1r_i", f32)
            be1rT = ld("be1r_i", f32)
            g2rT = ld("g2r_i", f32)
            be2rT = ld("be2r_i", f32)
            onesr = cst.tile([1, 1024], f32, name="onesr")
            nc.sync.dma_start(out=onesr, in_=CD["onesr_i"][:, :])
            ones3 = cst.tile([D, 1], f32, name="ones3")
            nc.vector.memset(ones3, 1.0)
            ones64 = cst.tile([64, 1], f32, name="ones64")
            nc.vector.memset(ones64, 1.0)

            bn1sc = cst.tile([128, 1], f32, name="bn1sc")
            bn1sh = cst.tile([128, 1], f32, name="bn1sh")
            bn2sc = cst.tile([128, 1], f32, name="bn2sc")
            bn2sh = cst.tile([128, 1], f32, name="bn2sh")

            # per-graph persistent (small) tiles
            posje = [big.tile([128, 480], f32, name=f"posje{g}") for g in range(GPC)]
            u1s = [big.tile([128, 512], f32r, name=f"u1s{g}") for g in range(GPC)]
            idx1s = [big.tile([128, 8 * K], u32, name=f"idx1s{g}") for g in range(GPC)]
            idx2s = [big.tile([128, 8 * K], u32, name=f"idx2s{g}") for g in range(GPC)]
            x1r = [big.tile([64, 1024], f32r, name=f"x1r{g}") for g in range(GPC)]
            x2r = [big.tile([128, 1024], f32r, name=f"x2r{g}") for g in range(GPC)]
            x1f = [big.tile([64, 1024], f32, name=f"x1f{g}") for g in range(GPC)]
            pooled4 = cst.tile([128, 32], f32, name="pooled4")
            s1acc = cst.tile([128, GPC], f32, name="s1acc")
            s1sq = cst.tile([128, GPC], f32, name="s1sq")
            s1pacc = cst.tile([128, GPC], f32, name="s1pacc")
            s2sq = cst.tile([128, GPC], f32, name="s2sq")
            for st_ in (s1acc, s1sq, s1pacc, s2sq):
                nc.vector.memset(st_, 0.0)

            P4a = [sm.tile([5, N], f32, name=f"P4a{g}", tag="P4a") for g in range(GPC)]
            P4b = [sm.tile([5, N], f32, name=f"P4b{g}", tag="P4b") for g in range(GPC)]

            def topk_chunks(src65a, src65b, idxout, extra_add):
                # exact top-20 per node: scores = -0.5*d^2, descending
                for c in range(8):
                    ps = psA.tile([128, N], f32, name="psd", tag="psa")
                    nc.tensor.matmul(ps[:, 0:512], src65a[:, 128 * c:128 * (c + 1)],
                                     src65b[:, 0:512], start=True, stop=True)
                    nc.tensor.matmul(ps[:, 512:1024], src65a[:, 128 * c:128 * (c + 1)],
                                     src65b[:, 512:1024], start=True, stop=True)
                    emb0 = wrk.tile([128, N], f32, name="emb0", tag="embA")
                    nc.scalar.activation(emb0, ps, AF.Copy)
                    m8 = wrk.tile([128, 24], f32, name="m8", tag="t24")
                    ix = wrk.tile([128, 24], u32, name="ix", tag="ix24")
                    nc.vector.max(out=m8[:, 0:8], in_=emb0)
                    nc.vector.max_index(out=ix[:, 0:8], in_max=m8[:, 0:8],
                                        in_values=emb0)
                    emb1 = wrk.tile([128, N], f32, name="emb1", tag="embB")
                    nc.vector.match_replace(out=emb1, in_to_replace=m8[:, 0:8],
                                            in_values=emb0, imm_value=NEG)
                    nc.vector.max(out=m8[:, 8:16], in_=emb1)
                    nc.vector.max_index(out=ix[:, 8:16], in_max=m8[:, 8:16],
                                        in_values=emb1)
                    emb2 = wrk.tile([128, N], f32, name="emb2", tag="embC")
                    nc.vector.match_replace(out=emb2, in_to_replace=m8[:, 8:16],
                                            in_values=emb1, imm_value=NEG)
                    nc.vector.max(out=m8[:, 16:24], in_=emb2)
                    nc.vector.max_index(out=ix[:, 16:24], in_max=m8[:, 16:24],
                                        in_values=emb2)
                    if extra_add:
                        nc.vector.tensor_scalar(idxout[:, K * c:K * (c + 1)],
                                                ix[:, 0:K], extra_add,
                                                scalar2=None, op0=ALU.add)
                    else:
                        nc.vector.tensor_copy(idxout[:, K * c:K * (c + 1)],
                                              ix[:, 0:K])

            # slice sl in [0,40): (c, q) = divmod(sl, 5); ranks 4q..4q+3 of chunk c
            # all MLP compute on partitions 0-63; groups of 2 slices -> [64,1024] psum
            def mat_h1(g, mode):
                for bt in range(5):
                    pst = psB.tile([96, 128], f32, name="pst", tag="psb")
                    nc.tensor.transpose(pst, posje[g][:, 96 * bt:96 * (bt + 1)], idT)
                    xtmp = wrk.tile([96, 128], f32r, name="xtmp", tag="xtmp")
                    nc.scalar.activation(xtmp, pst, AF.Copy)
                    piece = wrk.tile([3, 4096], f32r, name="piece", tag="piece", bufs=1)
                    for r3 in range(3):
                        nc.sync.dma_start(
                            out=piece[r3:r3 + 1, :].rearrange("o (t p) -> o t p", p=128),
                            in_=xtmp[r3:96:3, :])
                    for j in range(4 * bt, 4 * bt + 4):   # 1024-edge groups
                        ph = psC.tile([64, 1024], f32, name="ph", tag="psc")
                        for q_ in range(2):
                            sl = 2 * j + q_
                            cch = sl // 5
                            pcol = 512 * (sl - 8 * bt)
                            po = ph[:, 512 * q_:512 * (q_ + 1)]
                            nc.tensor.matmul(po, W1bT,
                                             piece[:, pcol:pcol + 512],
                                             start=True, stop=False)
                            nc.tensor.matmul(po, u1s[g][:, 64 * cch:64 * cch + 64],
                                             RT, start=False, stop=True)
                        if mode == 1:
                            sac = wrk.tile([64, 2], f32, name="sac", tag="sac")
                            d1 = wrk.tile([64, 1024], f32, name="d1", tag="d1")
                            nc.scalar.activation(d1, ph, AF.Copy,
                                                 accum_out=sac[:, 0:1])
                            d2 = wrk.tile([64, 1024], f32, name="d2", tag="d2")
                            nc.scalar.activation(d2, ph, AF.Square,
                                                 accum_out=sac[:, 1:2])
                            if j == 0:
                                nc.vector.tensor_copy(s1acc[0:64, g:g + 1], sac[:, 0:1])
                                nc.vector.tensor_copy(s1sq[0:64, g:g + 1], sac[:, 1:2])
                            else:
                                nc.vector.tensor_tensor(s1acc[0:64, g:g + 1],
                                                        s1acc[0:64, g:g + 1],
                                                        sac[:, 0:1], op=ALU.add)
                                nc.vector.tensor_tensor(s1sq[0:64, g:g + 1],
                                                        s1sq[0:64, g:g + 1],
                                                        sac[:, 1:2], op=ALU.add)
                        else:
                            sacp = wrk.tile([64, 1], f32, name="sacp", tag="sacp")
                            h1p = wrk.tile([64, 1024], f32r, name="h1p", tag="h1p")
                            nc.scalar.activation(h1p, ph, AF.Relu,
                                                 scale=bn1sc[0:64, 0:1],
                                                 bias=bn1sh[0:64, 0:1],
                                                 accum_out=sacp)
                            if mode == 2:
                                if j == 0:
                                    nc.vector.tensor_copy(s1pacc[0:64, g:g + 1], sacp)
                                else:
                                    nc.vector.tensor_tensor(s1pacc[0:64, g:g + 1],
                                                            s1pacc[0:64, g:g + 1],
                                                            sacp, op=ALU.add)
                            ph2 = psC.tile([64, 1024], f32, name="ph2", tag="psc")
                            nc.tensor.matmul(ph2[:, 0:512], W2T[0:64, :],
                                             h1p[:, 0:512], start=True, stop=True)
                            nc.tensor.matmul(ph2[:, 512:1024], W2T[0:64, :],
                                             h1p[:, 512:1024], start=True, stop=True)
                            if mode == 2:
                                sq2a = wrk.tile([64, 1], f32, name="sq2a", tag="sq2a")
                                d3 = wrk.tile([64, 1024], f32, name="d3", tag="d1")
                                nc.scalar.activation(d3, ph2, AF.Square,
                                                     accum_out=sq2a)
                                if j == 0:
                                    nc.vector.tensor_copy(s2sq[0:64, g:g + 1], sq2a)
                                else:
                                    nc.vector.tensor_tensor(s2sq[0:64, g:g + 1],
                                                            s2sq[0:64, g:g + 1],
                                                            sq2a, op=ALU.add)
                            else:
                                h2p = wrk.tile([64, 1024], f32r, name="h2p", tag="h1p")
                                nc.scalar.activation(h2p, ph2, AF.Relu,
                                                     scale=bn2sc[0:64, 0:1],
                                                     bias=bn2sh[0:64, 0:1])
                                ph3 = psC.tile([64, 1024], f32, name="ph3", tag="psc")
                                nc.tensor.matmul(ph3[:, 0:512], W3T[0:64, :],
                                                 h2p[:, 0:512], start=True, stop=True)
                                nc.tensor.matmul(ph3[:, 512:1024], W3T[0:64, :],
                                                 h2p[:, 512:1024],
                                                 start=True, stop=True)
                                h3t = wrk.tile([64, 1024], f32, name="h3t", tag="d2")
                                nc.scalar.activation(h3t, ph3, AF.Identity,
                                                     bias=b3cT[0:64, 0:1])
                                # streamed x1 partial reduce over the 2 slices
                                for q_ in range(2):
                                    sl = 2 * j + q_
                                    cch = sl // 5
                                    xcol = slice(128 * cch, 128 * (cch + 1))
                                    red = h3t[:, 512 * q_:512 * (q_ + 1)].rearrange(
                                        "z (rr p) -> z p rr", p=128)
                                    if sl % 5 == 0:
                                        nc.vector.tensor_reduce(
                                            out=x1f[g][:, xcol], in_=red,
                                            op=ALU.max, axis=mybir.AxisListType.X)
                                    else:
                                        xtm = wrk.tile([64, 128], f32, name="xtm",
                                                       tag="xtm")
                                        nc.vector.tensor_reduce(
                                            out=xtm, in_=red,
                                            op=ALU.max, axis=mybir.AxisListType.X)
                                        nc.vector.tensor_tensor(
                                            x1f[g][:, xcol], x1f[g][:, xcol],
                                            xtm, op=ALU.max)

            # ================= phase 1: kNN1, gathers, u1, stats1 =================
            for g in range(GPC):
                pg = pos4[N * g:N * (g + 1), :].rearrange("n c -> c n")
                nc.sync.dma_start(out=P4a[g][0:3, :], in_=pg)
                nc.sync.dma_start(out=P4b[g][0:3, :], in_=pg)
                nc.sync.dma_start(out=P4a[g][3:4, :], in_=onesr)
                nc.sync.dma_start(out=P4b[g][4:5, :], in_=onesr)
                psq = sm.tile([D, N], f32, name="psq", tag="psq")
                nc.scalar.activation(psq, P4a[g][0:3, :], AF.Square)
                ps1 = psA.tile([1, N], f32, name="ps1", tag="psa")
                nc.tensor.matmul(ps1[:, 0:512], ones3, psq[:, 0:512],
                                 start=True, stop=True)
                nc.tensor.matmul(ps1[:, 512:1024], ones3, psq[:, 512:1024],
                                 start=True, stop=True)
                msqrow = sm.tile([1, N], f32, name="msqrow", tag="msqrow")
                nc.scalar.activation(msqrow, ps1, AF.Copy, scale=-0.5)
                nc.sync.dma_start(out=P4b[g][3:4, :], in_=msqrow)
                nc.sync.dma_start(out=P4a[g][4:5, :], in_=msqrow)
                topk_chunks(P4a[g], P4b[g], idx1s[g], 1024 * g if g else None)

                for c in range(8):
                    pu = psB.tile([128, 64], f32, name="pu", tag="psb")
                    nc.tensor.matmul(pu, P4a[g][0:3, 128 * c:128 * (c + 1)],
                                     W1dT, start=True, stop=True)
                    nc.scalar.activation(u1s[g][:, 64 * c:64 * (c + 1)], pu, AF.Copy)

                for t in range(160):
                    c, r = divmod(t, K)
                    nc.gpsimd.indirect_dma_start(
                        out=posje[g][:, 3 * t:3 * t + 3], out_offset=None,
                        in_=pos4.ap(),
                        in_offset=IndirectOffsetOnAxis(
                            ap=idx1s[g][:, K * c + r:K * c + r + 1], axis=0))
                mat_h1(g, 1)

            # ================= AllReduce #1 =================
            def bn_allreduce(s_a, s_b, cc_in_t, cc_out_t, grow, berow, scol, shcol):
                stot = sm.tile([128, 2], f32, name="stot", tag="stot")
                nc.vector.tensor_reduce(out=stot[:, 0:1], in_=s_a,
                                        op=ALU.add, axis=mybir.AxisListType.X)
                nc.vector.tensor_reduce(out=stot[:, 1:2], in_=s_b,
                                        op=ALU.add, axis=mybir.AxisListType.X)
                pack = sm.tile([1, 128], f32, name="pack", tag="pack")
                nc.sync.dma_start(out=pack[:, 0:64], in_=stot[0:64, 0:1])
                nc.sync.dma_start(out=pack[:, 64:128], in_=stot[0:64, 1:2])
                nc.sync.dma_start(out=cc_in_t[:, :], in_=pack)
                nc.gpsimd.collective_compute(
                    "AllReduce", ALU.add, replica_groups=rg,
                    ins=[cc_in_t.ap().opt()], outs=[cc_out_t.ap().opt()])
                red = sm.tile([1, 128], f32, name="red", tag="red")
                nc.sync.dma_start(out=red, in_=cc_out_t[:, :])
                mean = sm.tile([1, 64], f32, name="mean", tag="mean")
                nc.vector.tensor_scalar(mean, red[:, 0:64], 1.0 / M_EDGES,
                                        scalar2=None, op0=ALU.mult)
                var = sm.tile([1, 64], f32, name="var", tag="var")
                nc.vector.tensor_scalar(var, red[:, 64:128], 1.0 / M_EDGES,
                                        scalar2=None, op0=ALU.mult)
                msq = sm.tile([1, 64], f32, name="msq", tag="msq")
                nc.vector.tensor_tensor(msq, mean, mean, op=ALU.mult)
                nc.vector.tensor_tensor(var, var, msq, op=ALU.subtract)
                nc.vector.tensor_scalar(var, var, EPS, scalar2=None, op0=ALU.add)
                rcp = sm.tile([1, 64], f32, name="rcp", tag="rcp")
                nc.vector.reciprocal(rcp, var)
                nc.scalar.activation(rcp, rcp, AF.Sqrt)
                scrow = sm.tile([1, 64], f32, name="scrow", tag="scrow")
                nc.vector.tensor_tensor(scrow, grow, rcp, op=ALU.mult)
                shrow = sm.tile([1, 64], f32, name="shrow", tag="shrow")
                nc.vector.tensor_tensor(shrow, scrow, mean, op=ALU.mult)
                nc.vector.tensor_tensor(shrow, berow, shrow, op=ALU.subtract)
                nc.sync.dma_start(out=scol[0:64, :], in_=scrow)
                nc.sync.dma_start(out=scol[64:128, :], in_=scrow)
                nc.sync.dma_start(out=shcol[0:64, :], in_=shrow)
                nc.sync.dma_start(out=shcol[64:128, :], in_=shrow)

            bn_allreduce(s1acc, s1sq, cc1_in, cc1_out, g1rT, be1rT, bn1sc, bn1sh)

            # ================= phase 2: stats2 =================
            for g in range(GPC):
                mat_h1(g, 2)
            s1pr = sm.tile([64, GPC], f32r, name="s1pr", tag="s1pr")
            nc.vector.tensor_copy(s1pr, s1pacc[0:64, :])
            ps2s = psB.tile([64, GPC], f32, name="ps2s", tag="psb")
            nc.tensor.matmul(ps2s, W2T[0:64, :], s1pr, start=True, stop=True)
            s2sum = sm.tile([128, GPC], f32, name="s2sum", tag="s2sum")
            nc.vector.memset(s2sum, 0.0)
            nc.scalar.activation(s2sum[0:64, :], ps2s, AF.Copy)

            bn_allreduce(s2sum, s2sq, cc2_in, cc2_out, g2rT, be2rT, bn2sc, bn2sh)

            # ====== phase 3+4 per graph: h3 -> x1; knn2; conv2; lin ======
            for g in range(GPC):
                mat_h1(g, 3)
                nc.vector.tensor_copy(x1r[g], x1f[g])

                # v2 node-major -> DRAM
                v2s = sm.tile([128, 1024], f32, name="v2s", tag="v2s")
                for c in range(8):
                    pv = psB.tile([128, 128], f32, name="pv", tag="psb")
                    nc.tensor.matmul(pv, x1r[g][:, 128 * c:128 * (c + 1)], Wc2bT,
                                     start=True, stop=True)
                    nc.scalar.activation(v2s[:, 128 * c:128 * (c + 1)], pv, AF.Copy)
                nc.sync.dma_start(
                    out=v2d[g].ap().rearrange("(c p) d -> p c d", p=128),
                    in_=v2s.rearrange("p (c d) -> p c d", c=8))

                # kNN2
                x1q = sm.tile([64, 1024], f32, name="x1q", tag="v2s")
                nc.scalar.activation(x1q, x1r[g].bitcast(f32), AF.Square)
                ps2 = psA.tile([1, N], f32, name="ps2", tag="psa")
                nc.tensor.matmul(ps2[:, 0:512], ones64, x1q[:, 0:512],
                                 start=True, stop=True)
                nc.tensor.matmul(ps2[:, 512:1024], ones64, x1q[:, 512:1024],
                                 start=True, stop=True)
                X65a = sm.tile([66, 1024], f32, name="X65a", tag="X65a")
                X65b = sm.tile([66, 1024], f32, name="X65b", tag="X65b")
                nc.scalar.activation(X65a[0:64, :], x1r[g].bitcast(f32), AF.Copy)
                nc.scalar.activation(X65b[0:64, :], x1r[g].bitcast(f32), AF.Copy)
                nc.sync.dma_start(out=X65a[64:65, :], in_=onesr)
                nc.sync.dma_start(out=X65b[65:66, :], in_=onesr)
                msq2row = sm.tile([1, N], f32, name="msq2row", tag="msqrow")
                nc.scalar.activation(msq2row, ps2, AF.Copy, scale=-0.5)
                nc.sync.dma_start(out=X65b[64:65, :], in_=msq2row)
                nc.sync.dma_start(out=X65a[65:66, :], in_=msq2row)
                topk_chunks(X65a, X65b, idx2s[g], None)

                # conv2 gather + max
                maxv2 = sm.tile([128, 1024], f32, name="maxv2", tag="v2s")
                for c in range(8):
                    gdest = wrk.tile([128, K * 128], f32, name="gdest", tag="gdest",
                                     bufs=1)
                    for r in range(K):
                        nc.gpsimd.indirect_dma_start(
                            out=gdest[:, 128 * r:128 * (r + 1)], out_offset=None,
                            in_=v2d[g].ap(),
                            in_offset=IndirectOffsetOnAxis(
                                ap=idx2s[g][:, K * c + r:K * c + r + 1], axis=0))
                    nc.vector.tensor_reduce(
                        out=maxv2[:, 128 * c:128 * (c + 1)],
                        in_=gdest.rearrange("p (r d) -> p d r", r=K),
                        op=ALU.max, axis=mybir.AxisListType.X)
                mvT = sm.tile([128, 1024], f32, name="mvT", tag="X65b")
                for c in range(8):
                    pt2 = psB.tile([128, 128], f32, name="pt2", tag="psb")
                    nc.tensor.transpose(pt2, maxv2[:, 128 * c:128 * (c + 1)], idT)
                    nc.scalar.activation(mvT[:, 128 * c:128 * (c + 1)], pt2, AF.Copy)
                u2s = sm.tile([128, 1024], f32, name="u2s", tag="X65a")
                for h2_ in range(2):
                    pu2 = psB.tile([128, 512], f32, name="pu2", tag="psb")
                    nc.tensor.matmul(pu2, Wc2dT, x1r[g][:, 512 * h2_:512 * (h2_ + 1)],
                                     start=True, stop=True)
                    nc.scalar.activation(u2s[:, 512 * h2_:512 * (h2_ + 1)], pu2,
                                         AF.Identity, bias=bc2cT[:, 0:1])
                nc.vector.tensor_tensor(x2r[g], u2s, mvT, op=ALU.add)

                # lin + maxpool
                for c in range(8):
                    pm = wrk.tile([128, 2], f32, name="pm", tag="pm")
                    for s_ in range(2):
                        pl = psB.tile([128, 512], f32, name="pl", tag="psb")
                        nc.tensor.matmul(pl, WlX1T[:, 128 * c:128 * (c + 1)],
                                         x1r[g][:, 512 * s_:512 * (s_ + 1)],
                                         start=True, stop=False)
                        nc.tensor.matmul(pl, WlX2T[:, 128 * c:128 * (c + 1)],
                                         x2r[g][:, 512 * s_:512 * (s_ + 1)],
                                         start=False, stop=True)
                        nc.vector.tensor_reduce(out=pm[:, s_:s_ + 1], in_=pl,
                                                op=ALU.max, axis=mybir.AxisListType.X)
                    nc.vector.tensor_tensor(pooled4[:, 4 * c + g:4 * c + g + 1],
                                            pm[:, 0:1], pm[:, 1:2], op=ALU.max)

            pooled4r = cst.tile([128, 32], f32r, name="pooled4r")
            for c in range(8):
                nc.vector.tensor_tensor(pooled4r[:, 4 * c:4 * (c + 1)],
                                        pooled4[:, 4 * c:4 * (c + 1)],
                                        blcT[:, c:c + 1].to_broadcast([128, GPC]),
                                        op=ALU.add)

            # ============ head MLP ============
            bm1cT = ld("bm1c_i", f32)
            bm2cT = ld("bm2c_i", f32)
            bm3cT = ld("bm3c_i", f32)

            hm1 = cst.tile([128, 4 * GPC], f32r, name="hm1")
            for cc in range(4):
                phm = psB.tile([128, GPC], f32, name="phm", tag="psb")
                for kk in range(8):
                    wslc = wrk.tile([128, 128], f32r, name="wslc", tag="wslc")
                    nc.sync.dma_start(
                        out=wslc,
                        in_=CD["Wm1_i"][:, 512 * kk + 128 * cc:
                                        512 * kk + 128 * (cc + 1)].bitcast(f32r))
                    nc.tensor.matmul(phm, wslc, pooled4r[:, 4 * kk:4 * (kk + 1)],
                                     start=(kk == 0), stop=(kk == 7))
                nc.scalar.activation(hm1[:, GPC * cc:GPC * (cc + 1)], phm, AF.Relu,
                                     bias=bm1cT[:, cc:cc + 1])
            hm2 = cst.tile([128, 2 * GPC], f32r, name="hm2")
            Wm2T = ld("Wm2_i", f32r)
            for cc in range(2):
                phm2 = psB.tile([128, GPC], f32, name="phm2", tag="psb")
                for kk in range(4):
                    nc.tensor.matmul(phm2,
                                     Wm2T[:, 256 * kk + 128 * cc:
                                          256 * kk + 128 * (cc + 1)],
                                     hm1[:, GPC * kk:GPC * (kk + 1)],
                                     start=(kk == 0), stop=(kk == 3))
                nc.scalar.activation(hm2[:, GPC * cc:GPC * (cc + 1)], phm2, AF.Relu,
                                     bias=bm2cT[:, cc:cc + 1])
            Wm3T = ld("Wm3_i", f32r)
            pho = psB.tile([40, GPC], f32, name="pho", tag="psb")
            for kk in range(2):
                nc.tensor.matmul(pho, Wm3T[:, 40 * kk:40 * (kk + 1)],
                                 hm2[:, GPC * kk:GPC * (kk + 1)],
                                 start=(kk == 0), stop=(kk == 1))
            outsb = cst.tile([40, GPC], f32, name="outsb")
            nc.scalar.activation(outsb, pho, AF.Identity, bias=bm3cT[:, 0:1])
            nc.sync.dma_start(out=out_t[:, :], in_=outsb)

    nc.compile()
    return nc


# ---------------- host wrapper ----------------
_CACHE = {}


def _consts_from_weights(W1, b1, g1, be1, W2, b2, g2, be2, W3, b3, Wc2, bc2,
                         Wl, bl, Wm1, bm1, Wm2, bm2, Wm3, bm3):
    f = np.float32
    W1 = np.asarray(W1, f); W3_ = np.asarray(W3, f); Wc2 = np.asarray(Wc2, f)
    Wl = np.asarray(Wl, f); Wm1 = np.asarray(Wm1, f); Wm2 = np.asarray(Wm2, f)
    Wm3 = np.asarray(Wm3, f)
    return dict(
        onesr_i=np.ones((1, 1024), f),
        Rsel=np.tile(np.eye(128, dtype=f), (1, 4)),
        ident=np.eye(128, dtype=f),
        W1b_i=W1[3:6].copy(), W1d_i=(W1[0:3] - W1[3:6]).copy(),
        W2_i=np.concatenate([np.asarray(W2, f)] * 2, 0),
        W3_i=np.concatenate([W3_] * 2, 0),
        Wc2d_i=(Wc2[0:64] - Wc2[64:128]).copy(), Wc2b_i=Wc2[64:128].copy(),
        WlX1_i=Wl[0:64].copy(), WlX2_i=Wl[64:192].copy(),
        Wm1_i=Wm1.reshape(8, 128, 512).transpose(1, 0, 2).reshape(128, 4096).copy(),
        Wm2_i=Wm2.reshape(4, 128, 256).transpose(1, 0, 2).reshape(128, 1024).copy(),
        Wm3_i=Wm3.reshape(2, 128, 40).transpose(1, 0, 2).reshape(128, 80).copy(),
        b3c_i=np.tile(np.asarray(b3, f), 2).reshape(128, 1),
        bc2c_i=np.asarray(bc2, f).reshape(128, 1),
        blc_i=np.asarray(bl, f).reshape(8, 128).T.copy(),
        bm1c_i=np.asarray(bm1, f).reshape(4, 128).T.copy(),
        bm2c_i=np.asarray(bm2, f).reshape(2, 128).T.copy(),
        bm3c_i=np.asarray(bm3, f).reshape(40, 1),
        g1r_i=np.asarray(g1, f).reshape(1, 64),
        be1r_i=np.asarray(be1, f).reshape(1, 64),
        g2r_i=np.asarray(g2, f).reshape(1, 64),
        be2r_i=np.asarray(be2, f).reshape(1, 64),
    )


def _weights_key(consts):
    h = hashlib.sha1()
    for k in sorted(consts):
        h.update(k.encode())
        h.update(np.ascontiguousarray(consts[k]).tobytes())
    return h.hexdigest()


def _build_runner(consts):
    import jax
    from concourse.bass2jax import (install_neuronx_cc_hook, _bass_exec_p,
                                    partition_id_tensor)
    from jax.sharding import Mesh, PartitionSpec
    from jax.experimental.shard_map import shard_map

    nc = _build(consts)
    install_neuronx_cc_hook()
    partition_name = nc.partition_id_tensor.name if nc.partition_id_tensor else None
    in_names, out_names, out_avals, zero_outs = [], [], [], []
    for alloc in nc.m.functions[0].allocations:
        if not isinstance(alloc, mybir.MemoryLocationSet):
            continue
        name = alloc.memorylocations[0].name
        if alloc.kind == "ExternalInput":
            if name != partition_name:
                in_names.append(name)
        elif alloc.kind == "ExternalOutput":
            out_names.append(name)
            shape = tuple(alloc.tensor_shape)
            dtype = mybir.dt.np(alloc.dtype)
            out_avals.append(jax.core.ShapedArray(shape, dtype))
            zero_outs.append(np.zeros(shape, dtype))
    n_params = len(in_names)
    n_outs = len(out_avals)
    all_in = list(in_names) + list(out_names)
    if partition_name is not None:
        all_in.append(partition_name)

    def _body(*args):
        operands = list(args)
        if partition_name is not None:
            operands.append(partition_id_tensor())
        return tuple(_bass_exec_p.bind(
            *operands, out_avals=tuple(out_avals), in_names=tuple(all_in),
            out_names=tuple(out_names), lowering_input_output_aliases=(),
            sim_require_finite=True, sim_require_nnan=True, nc=nc))

    donate = tuple(range(n_params, n_params + n_outs))
    devices = jax.devices()[:NCORES]
    mesh = Mesh(np.asarray(devices), ("core",))
    jitted = jax.jit(
        shard_map(_body, mesh=mesh,
                  in_specs=(PartitionSpec("core"),) * (n_params + n_outs),
                  out_specs=(PartitionSpec("core"),) * n_outs,
                  check_rep=False),
        donate_argnums=donate, keep_unused=True)

    def run(in_maps):
        import jax as _j
        concat_in = [np.concatenate([np.asarray(in_maps[c][n])
                                     for c in range(NCORES)], axis=0)
                     for n in in_names]
        concat_zero = [np.concatenate([z.copy() for _ in range(NCORES)], axis=0)
                       for z in zero_outs]
        outs = jitted(*concat_in, *concat_zero)
        _j.block_until_ready(outs)
        res = []
        for c in range(NCORES):
            d = {}
            for n, o, z in zip(out_names, outs, zero_outs):
                per = z.shape[0]
                d[n] = np.asarray(o[c * per:(c + 1) * per])
            res.append(d)
        return res

    return run


def _get_runner(inputs=None):
    if inputs is None:
        return _CACHE["run"]
    consts = _consts_from_weights(
        **{k: v for k, v in inputs.items() if k != "pos"})
    key = _weights_key(consts)
    if _CACHE.get("key") != key:
        _CACHE["run"] = _build_runner(consts)
        _CACHE["key"] = key
    return _CACHE["run"]


def _make_inputs(**inputs):
    pos = np.asarray(inputs["pos"], np.float32)
    return [{"pos4": pos[GPC * c:GPC * (c + 1)].reshape(GPC * N, D).copy()}
            for c in range(NCORES)]


def kernel(**inputs) -> np.ndarray:
    run = _get_runner(inputs)
    res = run(_make_inputs(**inputs))
    return np.concatenate([r["out"].T for r in res], axis=0)


if __name__ == "__main__":
    import reference as R  # only for a local smoke build
    import jax
    with jax.default_device(jax.devices("cpu")[0]):
        inputs = {k: np.asarray(v) for k, v in R.setup_inputs().items()}
    consts = _consts_from_weights(**{k: v for k, v in inputs.items()
                                     if k != "pos"})
    nc = _build(consts)
    print("built ok")


# revision 14
# speedup vs baseline: 16.6372x; 16.6372x over previous
"""DGCNN-alt Trainium2 kernel: 8-core data-parallel (4 graphs/core).

Self-contained: builds a Bass/Tile kernel (weights inlined into the NEFF as
constants), shards `pos` across 8 NeuronCores, runs via PJRT (axon), gathers
the full [32, 40] output.

kNN top-k is exact: f32 -d^2/2 scores + DVE max/match_replace/max_index
(top-24 descending, first-occurrence tie-break = reference tie-break).
"""
import sys
sys.path.insert(0, '/opt/trn_rl_repo')
import hashlib
import numpy as np

import concourse.bass as bass
from concourse import bacc
import concourse.mybir as mybir
from concourse.tile import TileContext
from concourse.bass import IndirectOffsetOnAxis

f32 = mybir.dt.float32
f32r = mybir.dt.float32r
u32 = mybir.dt.uint32
AF = mybir.ActivationFunctionType
ALU = mybir.AluOpType

# ---- problem constants ----
B, N, D, K = 32, 1024, 3, 20
GPC = 4                 # graphs per core
NCORES = 8
EPS = 1e-5
NEDGE = N * K           # 20480 edges/graph
M_EDGES = float(B * NEDGE)   # BN denominator over the full batch
NEG = -3.0e38


def _build(consts):
    nc = bacc.Bacc()

    # ---------------- I/O ----------------
    pos4 = nc.dram_tensor("pos4", [GPC * N, D], f32, kind="ExternalInput")
    # full gathered output (all 8 cores' [40, GPC] blocks), identical per core
    out_t = nc.dram_tensor("out", [1, 40 * GPC * NCORES], f32,
                           kind="ExternalOutput")

    # weights/constants baked into the executable
    CD = {name: nc.inline_tensor(np.ascontiguousarray(arr), name=name)
          for name, arr in consts.items()}

    # internal DRAM
    v2d = [nc.dram_tensor(f"v2d_{g}", [N, 128], f32) for g in range(GPC)]
    cc1_in = nc.dram_tensor("cc1_in", [1, 128], f32)
    cc1_out = nc.dram_tensor("cc1_out", [1, 128], f32, addr_space="Shared")
    cc2_in = nc.dram_tensor("cc2_in", [1, 128], f32)
    cc2_out = nc.dram_tensor("cc2_out", [1, 128], f32, addr_space="Shared")
    cg_in = nc.dram_tensor("cg_in", [1, 40 * GPC], f32)
    cg_out = nc.dram_tensor("cg_out", [1, 40 * GPC * NCORES], f32,
                            addr_space="Shared")
    rg = [list(range(NCORES))]

    with TileContext(nc) as tc:
        with tc.tile_pool(name="cst", bufs=1) as cst, \
             tc.tile_pool(name="big", bufs=1) as big, \
             tc.tile_pool(name="wrk", bufs=2) as wrk, \
             tc.tile_pool(name="sm", bufs=1) as sm, \
             tc.tile_pool(name="psA", bufs=1, space="PSUM") as psA, \
             tc.tile_pool(name="psB", bufs=2, space="PSUM") as psB, \
             tc.tile_pool(name="psC", bufs=2, space="PSUM") as psC:

            # ---------- load constants ----------
            def ld(name, dtype):
                arr = consts[name]
                t = cst.tile(list(arr.shape), dtype, name=name + "T")
                src = CD[name][:, :]
                if dtype != f32:
                    src = src.bitcast(dtype)
                nc.sync.dma_start(out=t, in_=src)
                return t

            RT = ld("Rsel", f32r)
            idT = ld("ident", f32)
            W1bT = ld("W1b_i", f32r)
            W1dT = ld("W1d_i", f32)
            W2T = ld("W2_i", f32r)
            W3T = ld("W3_i", f32r)
            Wc2dT = ld("Wc2d_i", f32r)
            Wc2bT = ld("Wc2b_i", f32r)
            WlX1T = ld("WlX1_i", f32r)
            WlX2T = ld("WlX2_i", f32r)
            b3cT = ld("b3c_i", f32)
            bc2cT = ld("bc2c_i", f32)
            blcT = ld("blc_i", f32)
            g1rT = ld("g1r_i", f32)
            be1rT = ld("be1r_i", f32)
            g2rT = ld("g2r_i", f32)
            be2rT = ld("be2r_i", f32)
            onesr = cst.tile([1, 1024], f32, name="onesr")
            nc.sync.dma_start(out=onesr, in_=CD["onesr_i"][:, :])
            ones3 = cst.tile([D, 1], f32, name="ones3")
            nc.vector.memset(ones3, 1.0)
            ones64 = cst.tile([64, 1], f32, name="ones64")
            nc.vector.memset(ones64, 1.0)

            bn1sc = cst.tile([128, 1], f32, name="bn1sc")
            bn1sh = cst.tile([128, 1], f32, name="bn1sh")
            bn2sc = cst.tile([128, 1], f32, name="bn2sc")
            bn2sh = cst.tile([128, 1], f32, name="bn2sh")

            # per-graph persistent (small) tiles
            posje = [big.tile([128, 480], f32, name=f"posje{g}") for g in range(GPC)]
            u1s = [big.tile([128, 512], f32r, name=f"u1s{g}") for g in range(GPC)]
            idx1s = [big.tile([128, 8 * K], u32, name=f"idx1s{g}") for g in range(GPC)]
            idx2s = [big.tile([128, 8 * K], u32, name=f"idx2s{g}") for g in range(GPC)]
            x1r = [big.tile([64, 1024], f32r, name=f"x1r{g}") for g in range(GPC)]
            x2r = [big.tile([128, 1024], f32r, name=f"x2r{g}") for g in range(GPC)]
            x1f = [big.tile([64, 1024], f32, name=f"x1f{g}") for g in range(GPC)]
            pooled4 = cst.tile([128, 32], f32, name="pooled4")
            s1acc = cst.tile([128, GPC], f32, name="s1acc")
            s1sq = cst.tile([128, GPC], f32, name="s1sq")
            s1pacc = cst.tile([128, GPC], f32, name="s1pacc")
            s2sq = cst.tile([128, GPC], f32, name="s2sq")
            for st_ in (s1acc, s1sq, s1pacc, s2sq):
                nc.vector.memset(st_, 0.0)

            P4a = [sm.tile([5, N], f32, name=f"P4a{g}", tag="P4a") for g in range(GPC)]
            P4b = [sm.tile([5, N], f32, name=f"P4b{g}", tag="P4b") for g in range(GPC)]

            def topk_chunks(src65a, src65b, idxout, extra_add):
                # exact top-20 per node: scores = -0.5*d^2, descending
                for c in range(8):
                    ps = psA.tile([128, N], f32, name="psd", tag="psa")
                    nc.tensor.matmul(ps[:, 0:512], src65a[:, 128 * c:128 * (c + 1)],
                                     src65b[:, 0:512], start=True, stop=True)
                    nc.tensor.matmul(ps[:, 512:1024], src65a[:, 128 * c:128 * (c + 1)],
                                     src65b[:, 512:1024], start=True, stop=True)
                    emb0 = wrk.tile([128, N], f32, name="emb0", tag="embA")
                    nc.scalar.activation(emb0, ps, AF.Copy)
                    m8 = wrk.tile([128, 24], f32, name="m8", tag="t24")
                    ix = wrk.tile([128, 24], u32, name="ix", tag="ix24")
                    nc.vector.max(out=m8[:, 0:8], in_=emb0)
                    nc.vector.max_index(out=ix[:, 0:8], in_max=m8[:, 0:8],
                                        in_values=emb0)
                    emb1 = wrk.tile([128, N], f32, name="emb1", tag="embB")
                    nc.vector.match_replace(out=emb1, in_to_replace=m8[:, 0:8],
                                            in_values=emb0, imm_value=NEG)
                    nc.vector.max(out=m8[:, 8:16], in_=emb1)
                    nc.vector.max_index(out=ix[:, 8:16], in_max=m8[:, 8:16],
                                        in_values=emb1)
                    emb2 = wrk.tile([128, N], f32, name="emb2", tag="embC")
                    nc.vector.match_replace(out=emb2, in_to_replace=m8[:, 8:16],
                                            in_values=emb1, imm_value=NEG)
                    nc.vector.max(out=m8[:, 16:24], in_=emb2)
                    nc.vector.max_index(out=ix[:, 16:24], in_max=m8[:, 16:24],
                                        in_values=emb2)
                    if extra_add:
                        nc.vector.tensor_scalar(idxout[:, K * c:K * (c + 1)],
                                                ix[:, 0:K], extra_add,
                                                scalar2=None, op0=ALU.add)
                    else:
                        nc.vector.tensor_copy(idxout[:, K * c:K * (c + 1)],
                                              ix[:, 0:K])

            # slice sl in [0,40): (c, q) = divmod(sl, 5); ranks 4q..4q+3 of chunk c
            # all MLP compute on partitions 0-63; groups of 2 slices -> [64,1024] psum
            def mat_h1(g, mode):
                for bt in range(5):
                    pst = psB.tile([96, 128], f32, name="pst", tag="psb")
                    nc.tensor.transpose(pst, posje[g][:, 96 * bt:96 * (bt + 1)], idT)
                    xtmp = wrk.tile([96, 128], f32r, name="xtmp", tag="xtmp")
                    nc.scalar.activation(xtmp, pst, AF.Copy)
                    piece = wrk.tile([3, 4096], f32r, name="piece", tag="piece", bufs=1)
                    for r3 in range(3):
                        nc.sync.dma_start(
                            out=piece[r3:r3 + 1, :].rearrange("o (t p) -> o t p", p=128),
                            in_=xtmp[r3:96:3, :])
                    for j in range(4 * bt, 4 * bt + 4):   # 1024-edge groups
                        ph = psC.tile([64, 1024], f32, name="ph", tag="psc")
                        for q_ in range(2):
                            sl = 2 * j + q_
                            cch = sl // 5
                            pcol = 512 * (sl - 8 * bt)
                            po = ph[:, 512 * q_:512 * (q_ + 1)]
                            nc.tensor.matmul(po, W1bT,
                                             piece[:, pcol:pcol + 512],
                                             start=True, stop=False)
                            nc.tensor.matmul(po, u1s[g][:, 64 * cch:64 * cch + 64],
                                             RT, start=False, stop=True)
                        if mode == 1:
                            sac = wrk.tile([64, 2], f32, name="sac", tag="sac")
                            d1 = wrk.tile([64, 1024], f32, name="d1", tag="d1")
                            nc.scalar.activation(d1, ph, AF.Copy,
                                                 accum_out=sac[:, 0:1])
                            d2 = wrk.tile([64, 1024], f32, name="d2", tag="d2")
                            nc.scalar.activation(d2, ph, AF.Square,
                                                 accum_out=sac[:, 1:2])
                            if j == 0:
                                nc.vector.tensor_copy(s1acc[0:64, g:g + 1], sac[:, 0:1])
                                nc.vector.tensor_copy(s1sq[0:64, g:g + 1], sac[:, 1:2])
                            else:
                                nc.vector.tensor_tensor(s1acc[0:64, g:g + 1],
                                                        s1acc[0:64, g:g + 1],
                                                        sac[:, 0:1], op=ALU.add)
                                nc.vector.tensor_tensor(s1sq[0:64, g:g + 1],
                                                        s1sq[0:64, g:g + 1],
                                                        sac[:, 1:2], op=ALU.add)
                        else:
                            sacp = wrk.tile([64, 1], f32, name="sacp", tag="sacp")
                            h1p = wrk.tile([64, 1024], f32r, name="h1p", tag="h1p")
                            nc.scalar.activation(h1p, ph, AF.Relu,
                                                 scale=bn1sc[0:64, 0:1],
                                                 bias=bn1sh[0:64, 0:1],
                                                 accum_out=sacp)
                            if mode == 2:
                                if j == 0:
                                    nc.vector.tensor_copy(s1pacc[0:64, g:g + 1], sacp)
                                else:
                                    nc.vector.tensor_tensor(s1pacc[0:64, g:g + 1],
                                                            s1pacc[0:64, g:g + 1],
                                                            sacp, op=ALU.add)
                            ph2 = psC.tile([64, 1024], f32, name="ph2", tag="psc")
                            nc.tensor.matmul(ph2[:, 0:512], W2T[0:64, :],
                                             h1p[:, 0:512], start=True, stop=True)
                            nc.tensor.matmul(ph2[:, 512:1024], W2T[0:64, :],
                                             h1p[:, 512:1024], start=True, stop=True)
                            if mode == 2:
                                sq2a = wrk.tile([64, 1], f32, name="sq2a", tag="sq2a")
                                d3 = wrk.tile([64, 1024], f32, name="d3", tag="d1")
                                nc.scalar.activation(d3, ph2, AF.Square,
                                                     accum_out=sq2a)
                                if j == 0:
                                    nc.vector.tensor_copy(s2sq[0:64, g:g + 1], sq2a)
                                else:
                                    nc.vector.tensor_tensor(s2sq[0:64, g:g + 1],
                                                            s2sq[0:64, g:g + 1],
                                                            sq2a, op=ALU.add)
                            else:
                                h2p = wrk.tile([64, 1024], f32r, name="h2p", tag="h1p")
                                nc.scalar.activation(h2p, ph2, AF.Relu,
                                                     scale=bn2sc[0:64, 0:1],
                                                     bias=bn2sh[0:64, 0:1])
                                ph3 = psC.tile([64, 1024], f32, name="ph3", tag="psc")
                                nc.tensor.matmul(ph3[:, 0:512], W3T[0:64, :],
                                                 h2p[:, 0:512], start=True, stop=True)
                                nc.tensor.matmul(ph3[:, 512:1024], W3T[0:64, :],
                                                 h2p[:, 512:1024],
                                                 start=True, stop=True)
                                h3t = wrk.tile([64, 1024], f32, name="h3t", tag="d2")
                                nc.scalar.activation(h3t, ph3, AF.Identity,
                                                     bias=b3cT[0:64, 0:1])
                                # streamed x1 partial reduce over the 2 slices
                                for q_ in range(2):
                                    sl = 2 * j + q_
                                    cch = sl // 5
                                    xcol = slice(128 * cch, 128 * (cch + 1))
                                    red = h3t[:, 512 * q_:512 * (q_ + 1)].rearrange(
                                        "z (rr p) -> z p rr", p=128)
                                    if sl % 5 == 0:
                                        nc.vector.tensor_reduce(
                                            out=x1f[g][:, xcol], in_=red,
                                            op=ALU.max, axis=mybir.AxisListType.X)
                                    else:
                                        xtm = wrk.tile([64, 128], f32, name="xtm",
                                                       tag="xtm")
                                        nc.vector.tensor_reduce(
                                            out=xtm, in_=red,
                                            op=ALU.max, axis=mybir.AxisListType.X)
                                        nc.vector.tensor_tensor(
                                            x1f[g][:, xcol], x1f[g][:, xcol],
                                            xtm, op=ALU.max)

            # ================= phase 1: kNN1, gathers, u1, stats1 =================
            for g in range(GPC):
                pg = pos4[N * g:N * (g + 1), :].rearrange("n c -> c n")
                nc.sync.dma_start(out=P4a[g][0:3, :], in_=pg)
                nc.sync.dma_start(out=P4b[g][0:3, :], in_=pg)
                nc.sync.dma_start(out=P4a[g][3:4, :], in_=onesr)
                nc.sync.dma_start(out=P4b[g][4:5, :], in_=onesr)
                psq = sm.tile([D, N], f32, name="psq", tag="psq")
                nc.scalar.activation(psq, P4a[g][0:3, :], AF.Square)
                ps1 = psA.tile([1, N], f32, name="ps1", tag="psa")
                nc.tensor.matmul(ps1[:, 0:512], ones3, psq[:, 0:512],
                                 start=True, stop=True)
                nc.tensor.matmul(ps1[:, 512:1024], ones3, psq[:, 512:1024],
                                 start=True, stop=True)
                msqrow = sm.tile([1, N], f32, name="msqrow", tag="msqrow")
                nc.scalar.activation(msqrow, ps1, AF.Copy, scale=-0.5)
                nc.sync.dma_start(out=P4b[g][3:4, :], in_=msqrow)
                nc.sync.dma_start(out=P4a[g][4:5, :], in_=msqrow)
                topk_chunks(P4a[g], P4b[g], idx1s[g], 1024 * g if g else None)

                for c in range(8):
                    pu = psB.tile([128, 64], f32, name="pu", tag="psb")
                    nc.tensor.matmul(pu, P4a[g][0:3, 128 * c:128 * (c + 1)],
                                     W1dT, start=True, stop=True)
                    nc.scalar.activation(u1s[g][:, 64 * c:64 * (c + 1)], pu, AF.Copy)

                for t in range(160):
                    c, r = divmod(t, K)
                    nc.gpsimd.indirect_dma_start(
                        out=posje[g][:, 3 * t:3 * t + 3], out_offset=None,
                        in_=pos4.ap(),
                        in_offset=IndirectOffsetOnAxis(
                            ap=idx1s[g][:, K * c + r:K * c + r + 1], axis=0))
                mat_h1(g, 1)

            # ================= AllReduce #1 =================
            def bn_allreduce(s_a, s_b, cc_in_t, cc_out_t, grow, berow, scol, shcol):
                stot = sm.tile([128, 2], f32, name="stot", tag="stot")
                nc.vector.tensor_reduce(out=stot[:, 0:1], in_=s_a,
                                        op=ALU.add, axis=mybir.AxisListType.X)
                nc.vector.tensor_reduce(out=stot[:, 1:2], in_=s_b,
                                        op=ALU.add, axis=mybir.AxisListType.X)
                pack = sm.tile([1, 128], f32, name="pack", tag="pack")
                nc.sync.dma_start(out=pack[:, 0:64], in_=stot[0:64, 0:1])
                nc.sync.dma_start(out=pack[:, 64:128], in_=stot[0:64, 1:2])
                nc.sync.dma_start(out=cc_in_t[:, :], in_=pack)
                nc.gpsimd.collective_compute(
                    "AllReduce", ALU.add, replica_groups=rg,
                    ins=[cc_in_t.ap().opt()], outs=[cc_out_t.ap().opt()])
                red = sm.tile([1, 128], f32, name="red", tag="red")
                nc.sync.dma_start(out=red, in_=cc_out_t[:, :])
                mean = sm.tile([1, 64], f32, name="mean", tag="mean")
                nc.vector.tensor_scalar(mean, red[:, 0:64], 1.0 / M_EDGES,
                                        scalar2=None, op0=ALU.mult)
                var = sm.tile([1, 64], f32, name="var", tag="var")
                nc.vector.tensor_scalar(var, red[:, 64:128], 1.0 / M_EDGES,
                                        scalar2=None, op0=ALU.mult)
                msq = sm.tile([1, 64], f32, name="msq", tag="msq")
                nc.vector.tensor_tensor(msq, mean, mean, op=ALU.mult)
                nc.vector.tensor_tensor(var, var, msq, op=ALU.subtract)
                nc.vector.tensor_scalar(var, var, EPS, scalar2=None, op0=ALU.add)
                rcp = sm.tile([1, 64], f32, name="rcp", tag="rcp")
                nc.vector.reciprocal(rcp, var)
                nc.scalar.activation(rcp, rcp, AF.Sqrt)
                scrow = sm.tile([1, 64], f32, name="scrow", tag="scrow")
                nc.vector.tensor_tensor(scrow, grow, rcp, op=ALU.mult)
                shrow = sm.tile([1, 64], f32, name="shrow", tag="shrow")
                nc.vector.tensor_tensor(shrow, scrow, mean, op=ALU.mult)
                nc.vector.tensor_tensor(shrow, berow, shrow, op=ALU.subtract)
                nc.sync.dma_start(out=scol[0:64, :], in_=scrow)
                nc.sync.dma_start(out=scol[64:128, :], in_=scrow)
                nc.sync.dma_start(out=shcol[0:64, :], in_=shrow)
                nc.sync.dma_start(out=shcol[64:128, :], in_=shrow)

            bn_allreduce(s1acc, s1sq, cc1_in, cc1_out, g1rT, be1rT, bn1sc, bn1sh)

            # ================= phase 2: stats2 =================
            for g in range(GPC):
                mat_h1(g, 2)
            s1pr = sm.tile([64, GPC], f32r, name="s1pr", tag="s1pr")
            nc.vector.tensor_copy(s1pr, s1pacc[0:64, :])
            ps2s = psB.tile([64, GPC], f32, name="ps2s", tag="psb")
            nc.tensor.matmul(ps2s, W2T[0:64, :], s1pr, start=True, stop=True)
            s2sum = sm.tile([128, GPC], f32, name="s2sum", tag="s2sum")
            nc.vector.memset(s2sum, 0.0)
            nc.scalar.activation(s2sum[0:64, :], ps2s, AF.Copy)

            bn_allreduce(s2sum, s2sq, cc2_in, cc2_out, g2rT, be2rT, bn2sc, bn2sh)

            # ====== phase 3+4 per graph: h3 -> x1; knn2; conv2; lin ======
            for g in range(GPC):
                mat_h1(g, 3)
                nc.vector.tensor_copy(x1r[g], x1f[g])

                # v2 node-major -> DRAM
                v2s = sm.tile([128, 1024], f32, name="v2s", tag="v2s")
                for c in range(8):
                    pv = psB.tile([128, 128], f32, name="pv", tag="psb")
                    nc.tensor.matmul(pv, x1r[g][:, 128 * c:128 * (c + 1)], Wc2bT,
                                     start=True, stop=True)
                    nc.scalar.activation(v2s[:, 128 * c:128 * (c + 1)], pv, AF.Copy)
                nc.sync.dma_start(
                    out=v2d[g].ap().rearrange("(c p) d -> p c d", p=128),
                    in_=v2s.rearrange("p (c d) -> p c d", c=8))

                # kNN2
                x1q = sm.tile([64, 1024], f32, name="x1q", tag="v2s")
                nc.scalar.activation(x1q, x1r[g].bitcast(f32), AF.Square)
                ps2 = psA.tile([1, N], f32, name="ps2", tag="psa")
                nc.tensor.matmul(ps2[:, 0:512], ones64, x1q[:, 0:512],
                                 start=True, stop=True)
                nc.tensor.matmul(ps2[:, 512:1024], ones64, x1q[:, 512:1024],
                                 start=True, stop=True)
                X65a = sm.tile([66, 1024], f32, name="X65a", tag="X65a")
                X65b = sm.tile([66, 1024], f32, name="X65b", tag="X65b")
                nc.scalar.activation(X65a[0:64, :], x1r[g].bitcast(f32), AF.Copy)
                nc.scalar.activation(X65b[0:64, :], x1r[g].bitcast(f32), AF.Copy)
                nc.sync.dma_start(out=X65a[64:65, :], in_=onesr)
                nc.sync.dma_start(out=X65b[65:66, :], in_=onesr)
                msq2row = sm.tile([1, N], f32, name="msq2row", tag="msqrow")
                nc.scalar.activation(msq2row, ps2, AF.Copy, scale=-0.5)
                nc.sync.dma_start(out=X65b[64:65, :], in_=msq2row)
                nc.sync.dma_start(out=X65a[65:66, :], in_=msq2row)
                topk_chunks(X65a, X65b, idx2s[g], None)

                # conv2 gather + max
                maxv2 = sm.tile([128, 1024], f32, name="maxv2", tag="v2s")
                for c in range(8):
                    gdest = wrk.tile([128, K * 128], f32, name="gdest", tag="gdest",
                                     bufs=1)
                    for r in range(K):
                        nc.gpsimd.indirect_dma_start(
                            out=gdest[:, 128 * r:128 * (r + 1)], out_offset=None,
                            in_=v2d[g].ap(),
                            in_offset=IndirectOffsetOnAxis(
                                ap=idx2s[g][:, K * c + r:K * c + r + 1], axis=0))
                    nc.vector.tensor_reduce(
                        out=maxv2[:, 128 * c:128 * (c + 1)],
                        in_=gdest.rearrange("p (r d) -> p d r", r=K),
                        op=ALU.max, axis=mybir.AxisListType.X)
                mvT = sm.tile([128, 1024], f32, name="mvT", tag="X65b")
                for c in range(8):
                    pt2 = psB.tile([128, 128], f32, name="pt2", tag="psb")
                    nc.tensor.transpose(pt2, maxv2[:, 128 * c:128 * (c + 1)], idT)
                    nc.scalar.activation(mvT[:, 128 * c:128 * (c + 1)], pt2, AF.Copy)
                u2s = sm.tile([128, 1024], f32, name="u2s", tag="X65a")
                for h2_ in range(2):
                    pu2 = psB.tile([128, 512], f32, name="pu2", tag="psb")
                    nc.tensor.matmul(pu2, Wc2dT, x1r[g][:, 512 * h2_:512 * (h2_ + 1)],
                                     start=True, stop=True)
                    nc.scalar.activation(u2s[:, 512 * h2_:512 * (h2_ + 1)], pu2,
                                         AF.Identity, bias=bc2cT[:, 0:1])
                nc.vector.tensor_tensor(x2r[g], u2s, mvT, op=ALU.add)

                # lin + maxpool
                for c in range(8):
                    pm = wrk.tile([128, 2], f32, name="pm", tag="pm")
                    for s_ in range(2):
                        pl = psB.tile([128, 512], f32, name="pl", tag="psb")
                        nc.tensor.matmul(pl, WlX1T[:, 128 * c:128 * (c + 1)],
                                         x1r[g][:, 512 * s_:512 * (s_ + 1)],
                                         start=True, stop=False)
                        nc.tensor.matmul(pl, WlX2T[:, 128 * c:128 * (c + 1)],
                                         x2r[g][:, 512 * s_:512 * (s_ + 1)],
                                         start=False, stop=True)
                        nc.vector.tensor_reduce(out=pm[:, s_:s_ + 1], in_=pl,
                                                op=ALU.max, axis=mybir.AxisListType.X)
                    nc.vector.tensor_tensor(pooled4[:, 4 * c + g:4 * c + g + 1],
                                            pm[:, 0:1], pm[:, 1:2], op=ALU.max)

            pooled4r = cst.tile([128, 32], f32r, name="pooled4r")
            for c in range(8):
                nc.vector.tensor_tensor(pooled4r[:, 4 * c:4 * (c + 1)],
                                        pooled4[:, 4 * c:4 * (c + 1)],
                                        blcT[:, c:c + 1].to_broadcast([128, GPC]),
                                        op=ALU.add)

            # ============ head MLP ============
            bm1cT = ld("bm1c_i", f32)
            bm2cT = ld("bm2c_i", f32)
            bm3cT = ld("bm3c_i", f32)

            hm1 = cst.tile([128, 4 * GPC], f32r, name="hm1")
            for cc in range(4):
                phm = psB.tile([128, GPC], f32, name="phm", tag="psb")
                for kk in range(8):
                    wslc = wrk.tile([128, 128], f32r, name="wslc", tag="wslc")
                    nc.sync.dma_start(
                        out=wslc,
                        in_=CD["Wm1_i"][:, 512 * kk + 128 * cc:
                                        512 * kk + 128 * (cc + 1)].bitcast(f32r))
                    nc.tensor.matmul(phm, wslc, pooled4r[:, 4 * kk:4 * (kk + 1)],
                                     start=(kk == 0), stop=(kk == 7))
                nc.scalar.activation(hm1[:, GPC * cc:GPC * (cc + 1)], phm, AF.Relu,
                                     bias=bm1cT[:, cc:cc + 1])
            hm2 = cst.tile([128, 2 * GPC], f32r, name="hm2")
            Wm2T = ld("Wm2_i", f32r)
            for cc in range(2):
                phm2 = psB.tile([128, GPC], f32, name="phm2", tag="psb")
                for kk in range(4):
                    nc.tensor.matmul(phm2,
                                     Wm2T[:, 256 * kk + 128 * cc:
                                          256 * kk + 128 * (cc + 1)],
                                     hm1[:, GPC * kk:GPC * (kk + 1)],
                                     start=(kk == 0), stop=(kk == 3))
                nc.scalar.activation(hm2[:, GPC * cc:GPC * (cc + 1)], phm2, AF.Relu,
                                     bias=bm2cT[:, cc:cc + 1])
            Wm3T = ld("Wm3_i", f32r)
            pho = psB.tile([40, GPC], f32, name="pho", tag="psb")
            for kk in range(2):
                nc.tensor.matmul(pho, Wm3T[:, 40 * kk:40 * (kk + 1)],
                                 hm2[:, GPC * kk:GPC * (kk + 1)],
                                 start=(kk == 0), stop=(kk == 1))
            outsb = cst.tile([40, GPC], f32, name="outsb")
            nc.scalar.activation(outsb, pho, AF.Identity, bias=bm3cT[:, 0:1])
            nc.sync.dma_start(out=cg_in.ap().rearrange("o (p q) -> (o p) q",
                                                       p=40),
                              in_=outsb)
            nc.gpsimd.collective_compute(
                "AllGather", ALU.bypass, replica_groups=rg,
                ins=[cg_in.ap().opt()], outs=[cg_out.ap().opt()])
            allout = cst.tile([1, 40 * GPC * NCORES], f32, name="allout")
            nc.sync.dma_start(out=allout, in_=cg_out[:, :])
            nc.sync.dma_start(out=out_t[:, :], in_=allout)

    nc.compile()
    return nc


# ---------------- host wrapper ----------------
_CACHE = {}


def _consts_from_weights(W1, b1, g1, be1, W2, b2, g2, be2, W3, b3, Wc2, bc2,
                         Wl, bl, Wm1, bm1, Wm2, bm2, Wm3, bm3):
    f = np.float32
    W1 = np.asarray(W1, f); W3_ = np.asarray(W3, f); Wc2 = np.asarray(Wc2, f)
    Wl = np.asarray(Wl, f); Wm1 = np.asarray(Wm1, f); Wm2 = np.asarray(Wm2, f)
    Wm3 = np.asarray(Wm3, f)
    return dict(
        onesr_i=np.ones((1, 1024), f),
        Rsel=np.tile(np.eye(128, dtype=f), (1, 4)),
        ident=np.eye(128, dtype=f),
        W1b_i=W1[3:6].copy(), W1d_i=(W1[0:3] - W1[3:6]).copy(),
        W2_i=np.concatenate([np.asarray(W2, f)] * 2, 0),
        W3_i=np.concatenate([W3_] * 2, 0),
        Wc2d_i=(Wc2[0:64] - Wc2[64:128]).copy(), Wc2b_i=Wc2[64:128].copy(),
        WlX1_i=Wl[0:64].copy(), WlX2_i=Wl[64:192].copy(),
        Wm1_i=Wm1.reshape(8, 128, 512).transpose(1, 0, 2).reshape(128, 4096).copy(),
        Wm2_i=Wm2.reshape(4, 128, 256).transpose(1, 0, 2).reshape(128, 1024).copy(),
        Wm3_i=Wm3.reshape(2, 128, 40).transpose(1, 0, 2).reshape(128, 80).copy(),
        b3c_i=np.tile(np.asarray(b3, f), 2).reshape(128, 1),
        bc2c_i=np.asarray(bc2, f).reshape(128, 1),
        blc_i=np.asarray(bl, f).reshape(8, 128).T.copy(),
        bm1c_i=np.asarray(bm1, f).reshape(4, 128).T.copy(),
        bm2c_i=np.asarray(bm2, f).reshape(2, 128).T.copy(),
        bm3c_i=np.asarray(bm3, f).reshape(40, 1),
        g1r_i=np.asarray(g1, f).reshape(1, 64),
        be1r_i=np.asarray(be1, f).reshape(1, 64),
        g2r_i=np.asarray(g2, f).reshape(1, 64),
        be2r_i=np.asarray(be2, f).reshape(1, 64),
    )


def _weights_key(consts):
    h = hashlib.sha1()
    for k in sorted(consts):
        h.update(k.encode())
        h.update(np.ascontiguousarray(consts[k]).tobytes())
    return h.hexdigest()


def _build_runner(consts):
    import jax
    from concourse.bass2jax import (install_neuronx_cc_hook, _bass_exec_p,
                                    partition_id_tensor)
    from jax.sharding import Mesh, PartitionSpec
    from jax.experimental.shard_map import shard_map

    nc = _build(consts)
    install_neuronx_cc_hook()
    partition_name = nc.partition_id_tensor.name if nc.partition_id_tensor else None
    in_names, out_names, out_avals, zero_outs = [], [], [], []
    for alloc in nc.m.functions[0].allocations:
        if not isinstance(alloc, mybir.MemoryLocationSet):
            continue
        name = alloc.memorylocations[0].name
        if alloc.kind == "ExternalInput":
            if name != partition_name:
                in_names.append(name)
        elif alloc.kind == "ExternalOutput":
            out_names.append(name)
            shape = tuple(alloc.tensor_shape)
            dtype = mybir.dt.np(alloc.dtype)
            out_avals.append(jax.core.ShapedArray(shape, dtype))
            zero_outs.append(np.zeros(shape, dtype))
    n_params = len(in_names)
    n_outs = len(out_avals)
    all_in = list(in_names) + list(out_names)
    if partition_name is not None:
        all_in.append(partition_name)

    def _body(*args):
        operands = list(args)
        if partition_name is not None:
            operands.append(partition_id_tensor())
        return tuple(_bass_exec_p.bind(
            *operands, out_avals=tuple(out_avals), in_names=tuple(all_in),
            out_names=tuple(out_names), lowering_input_output_aliases=(),
            sim_require_finite=True, sim_require_nnan=True, nc=nc))

    donate = tuple(range(n_params, n_params + n_outs))
    devices = jax.devices()[:NCORES]
    mesh = Mesh(np.asarray(devices), ("core",))
    jitted = jax.jit(
        shard_map(_body, mesh=mesh,
                  in_specs=(PartitionSpec("core"),) * (n_params + n_outs),
                  out_specs=(PartitionSpec("core"),) * n_outs,
                  check_rep=False),
        donate_argnums=donate, keep_unused=True)

    def run(in_maps):
        concat_in = [np.concatenate([np.asarray(in_maps[c][n])
                                     for c in range(NCORES)], axis=0)
                     for n in in_names]
        concat_zero = [np.concatenate([z.copy() for _ in range(NCORES)], axis=0)
                       for z in zero_outs]
        outs = jitted(*concat_in, *concat_zero)
        # every core holds the full gathered result; fetch core 0's shard only
        full = np.asarray(outs[0].addressable_shards[0].data)
        blocks = full.reshape(NCORES, 40, GPC)
        return [{"out": blocks[c]} for c in range(NCORES)]

    _CACHE["jitted"] = jitted
    _CACHE["in_names"] = in_names
    _CACHE["out_names"] = out_names
    _CACHE["zero_outs"] = zero_outs
    _CACHE["mesh"] = mesh
    return run


def _get_runner(inputs=None):
    if inputs is None:
        return _CACHE["run"]
    idkey = tuple(sorted((k, id(v), np.shape(v))
                         for k, v in inputs.items() if k != "pos"))
    if _CACHE.get("idkey") == idkey and "run" in _CACHE:
        return _CACHE["run"]
    consts = _consts_from_weights(
        **{k: v for k, v in inputs.items() if k != "pos"})
    key = _weights_key(consts)
    if _CACHE.get("key") != key:
        _CACHE["run"] = _build_runner(consts)
        _CACHE["key"] = key
    _CACHE["idkey"] = idkey
    return _CACHE["run"]


def _make_inputs(**inputs):
    pos = np.asarray(inputs["pos"], np.float32)
    return [{"pos4": pos[GPC * c:GPC * (c + 1)].reshape(GPC * N, D).copy()}
            for c in range(NCORES)]


def kernel(**inputs) -> np.ndarray:
    run = _get_runner(inputs)
    res = run(_make_inputs(**inputs))
    return np.concatenate([r["out"].T for r in res], axis=0)


if __name__ == "__main__":
    import reference as R  # only for a local smoke build
    import jax
    with jax.default_device(jax.devices("cpu")[0]):
        inputs = {k: np.asarray(v) for k, v in R.setup_inputs().items()}
    consts = _consts_from_weights(**{k: v for k, v in inputs.items()
                                     if k != "pos"})
    nc = _build(consts)
    print("built ok")
